# revision 5
# baseline (speedup 1.0000x reference)
"""Multi-head self-attention (B=1, S=2048, E=1024, H=16, D=64) on 8 NeuronCores.

Tensor-parallel by head: core c owns heads {2c, 2c+1}. Each core computes
  qT/kT = (w_q/k^T @ qkv^T + b)        [128, S]   (2 heads x 64 stacked)
  v     = qkv @ w_v + b_v              [S, 128]
  scoresT_h = k_h @ q_h^T              [S(t), S(s)] per head (K=64 matmuls,
                                        both heads concurrent via PE row groups)
  expT_h = exp(scoresT / 8)            (no max-subtraction: scores ~ N(0,1))
  [outT_h; Z_h] = [v_h | 1]^T @ expT_h [65, S]    (ones column -> softmax denom)
  attn_outT = outT_h * (1/Z_h)         [128, S]
  partial = attn_outT^T @ w_out_rows   [S, E]
Host sums the 8 partials and adds b_out.
"""

import os
import sys

import numpy as np

try:
    import concourse.bass as bass  # noqa: F401
except ImportError:
    sys.path.insert(0, "/opt/trn_rl_repo")

import ml_dtypes

import concourse.bass as bass
import concourse.mybir as mybir
import concourse.tile as tile
from concourse import bacc, bass_utils

S = 2048
E = 1024
H = 16
D = 64
NCORE = 8
HC = H // NCORE          # heads per core = 2
J = HC * D               # local feature width = 128
KE = E // 128            # contraction tiles for in_proj = 8
NT = S // 128            # 128-row tiles of the sequence = 16
CH = 512                 # free-dim chunk (one PSUM bank)
NCH = S // CH            # s-chunks = 4
SCALE = 1.0 / np.sqrt(D)

MM_DT = mybir.dt.bfloat16
MM_NP = ml_dtypes.bfloat16

_cached = None


def _build():
    f32 = mybir.dt.float32
    nc = bacc.Bacc("TRN2", target_bir_lowering=False)

    d_qkvT = nc.dram_tensor("qkvT", (E, S), MM_DT, kind="ExternalInput")
    d_wq = nc.dram_tensor("wq", (E, J), MM_DT, kind="ExternalInput")
    d_wk = nc.dram_tensor("wk", (E, J), MM_DT, kind="ExternalInput")
    d_wv = nc.dram_tensor("wv", (E, J), MM_DT, kind="ExternalInput")
    d_bq = nc.dram_tensor("bq", (J, 1), f32, kind="ExternalInput")
    d_bk = nc.dram_tensor("bk", (J, 1), f32, kind="ExternalInput")
    d_bv = nc.dram_tensor("bv", (128, J), f32, kind="ExternalInput")
    d_wout = nc.dram_tensor("wout", (J, E), MM_DT, kind="ExternalInput")
    d_out = nc.dram_tensor("partial", (S, E), f32, kind="ExternalOutput")

    with tile.TileContext(nc) as tc:
        with (
            tc.tile_pool(name="persist", bufs=1) as persist,
            tc.tile_pool(name="expp", bufs=2) as expp,
            tc.tile_pool(name="outp", bufs=3) as outp,
            tc.tile_pool(name="small", bufs=4) as small,
            tc.tile_pool(name="ps_sc", bufs=3, space="PSUM") as ps_sc,
            tc.tile_pool(name="ps_mm", bufs=2, space="PSUM") as ps_mm,
            tc.tile_pool(name="ps_av", bufs=1, space="PSUM") as ps_av,
        ):
            # ---- persistent SBUF ----
            sb_qkvT = persist.tile([128, KE, S], MM_DT)
            sb_wq = persist.tile([128, KE, J], MM_DT)
            sb_wk = persist.tile([128, KE, J], MM_DT)
            sb_wv = persist.tile([128, KE, J], MM_DT)
            sb_bq = persist.tile([J, 1], f32)
            sb_bk = persist.tile([J, 1], f32)
            sb_bv = persist.tile([128, J], f32)
            sb_wout = persist.tile([J, E], MM_DT)
            sb_qT = persist.tile([J, S], MM_DT)
            sb_kT = persist.tile([J, S], MM_DT)
            # v augmented per head with a 64-wide ones block: the AV matmul
            # then yields Z replicated on partitions 64..127 (broadcast for free)
            sb_v = persist.tile([128, NT, HC * 2 * D], MM_DT)
            sb_attnT = persist.tile([J, S], MM_DT)

            for k in range(KE):
                nc.sync.dma_start(
                    out=sb_qkvT[:, k, :], in_=d_qkvT[k * 128 : (k + 1) * 128, :]
                )
            nc.sync.dma_start(
                out=sb_wq[:], in_=d_wq.rearrange("(k p) m -> p k m", p=128)
            )
            nc.sync.dma_start(
                out=sb_wk[:], in_=d_wk.rearrange("(k p) m -> p k m", p=128)
            )
            nc.sync.dma_start(
                out=sb_wv[:], in_=d_wv.rearrange("(k p) m -> p k m", p=128)
            )
            nc.sync.dma_start(out=sb_bq[:], in_=d_bq[:])
            nc.sync.dma_start(out=sb_bk[:], in_=d_bk[:])
            nc.sync.dma_start(out=sb_bv[:], in_=d_bv[:])
            nc.sync.dma_start(out=sb_wout[:], in_=d_wout[:])

            nc.vector.memset(sb_v[:, :, D : 2 * D], 1.0)
            nc.vector.memset(sb_v[:, :, 3 * D :], 1.0)

            # ---- in_proj: kT and qT, [J, S] with head-stacked partitions ----
            for name, sb_w, sb_b, sb_dst in (
                ("k", sb_wk, sb_bk, sb_kT),
                ("q", sb_wq, sb_bq, sb_qT),
            ):
                for c in range(NCH):
                    ps = ps_mm.tile([128, CH], f32, tag="ps_b")
                    for k in range(KE):
                        nc.tensor.matmul(
                            ps[:],
                            sb_w[:, k, :],
                            sb_qkvT[:, k, c * CH : (c + 1) * CH],
                            start=(k == 0),
                            stop=(k == KE - 1),
                        )
                    nc.vector.tensor_scalar_add(
                        sb_dst[:, c * CH : (c + 1) * CH], ps[:], sb_b[:]
                    )

            # ---- in_proj: v, [S, J] (t on partitions) ----
            for t in range(NT):
                ps = ps_sc.tile([128, CH], f32, tag="ps_s")
                for k in range(KE):
                    nc.tensor.matmul(
                        ps[:, :J],
                        sb_qkvT[:, k, t * 128 : (t + 1) * 128],
                        sb_wv[:, k, :],
                        start=(k == 0),
                        stop=(k == KE - 1),
                    )
                for h in range(HC):
                    nc.vector.tensor_add(
                        sb_v[:, t, h * 2 * D : h * 2 * D + D],
                        ps[:, h * D : (h + 1) * D],
                        sb_bv[:, h * D : (h + 1) * D],
                    )

            # ---- attention, chunk by chunk over s ----
            for c in range(NCH):
                s_sl = slice(c * CH, (c + 1) * CH)
                exps = [
                    expp.tile([128, NT, CH], MM_DT, tag=f"exp{h}", name=f"exp{h}") for h in range(HC)
                ]
                for t in range(NT):
                    for h in range(HC):
                        hd = slice(h * D, (h + 1) * D)
                        ps_s = ps_sc.tile([128, CH], f32, tag="ps_s")
                        nc.tensor.matmul(
                            ps_s[:],
                            sb_kT[hd, t * 128 : (t + 1) * 128],
                            sb_qT[hd, s_sl],
                            start=True,
                            stop=True,
                        )
                        nc.scalar.activation(
                            exps[h][:, t, :],
                            ps_s[:],
                            mybir.ActivationFunctionType.Exp,
                            scale=float(SCALE),
                        )

                ps_o = [
                    ps_av.tile([128, CH], f32, tag=f"ps_av{h}", name=f"ps_av{h}") for h in range(HC)
                ]
                for t in range(NT):
                    for h in range(HC):
                        nc.tensor.matmul(
                            ps_o[h][:],
                            sb_v[:, t, h * 2 * D : (h + 1) * 2 * D],
                            exps[h][:, t, :],
                            start=(t == 0),
                            stop=(t == NT - 1),
                        )

                # normalize: attn_outT[h] = outT[h] * (1/Z[h]) broadcast over d
                for h in range(HC):
                    hd = slice(h * D, (h + 1) * D)
                    rbc = small.tile([D, CH], f32, tag="rbc")
                    nc.vector.reciprocal(rbc[:], ps_o[h][D : 2 * D, :])
                    nc.vector.tensor_mul(
                        sb_attnT[hd, s_sl], ps_o[h][:D, :], rbc[:]
                    )

                # ---- out_proj for this chunk ----
                for st in range(CH // 128):
                    t = c * (CH // 128) + st
                    sb_out = outp.tile([128, E], f32, tag="out")
                    for ec in range(E // CH):
                        ps_p = ps_mm.tile([128, CH], f32, tag="ps_b")
                        nc.tensor.matmul(
                            ps_p[:],
                            sb_attnT[:, t * 128 : (t + 1) * 128],
                            sb_wout[:, ec * CH : (ec + 1) * CH],
                            start=True,
                            stop=True,
                        )
                        nc.vector.tensor_copy(
                            sb_out[:, ec * CH : (ec + 1) * CH], ps_p[:]
                        )
                    nc.sync.dma_start(
                        out=d_out[t * 128 : (t + 1) * 128, :], in_=sb_out[:]
                    )

    nc.finalize()
    return nc


def _prep_inputs(qkv, w_in, b_in, w_out):
    qkv2 = np.asarray(qkv, np.float32).reshape(S, E)
    qkvT = np.ascontiguousarray(qkv2.T).astype(MM_NP)
    w_in = np.asarray(w_in, np.float32)
    b_in = np.asarray(b_in, np.float32)
    w_out = np.asarray(w_out, np.float32)
    in_maps = []
    for c in range(NCORE):
        cols = slice(c * J, c * J + J)
        in_maps.append(
            {
                "qkvT": qkvT,
                "wq": np.ascontiguousarray(w_in[:, :E][:, cols]).astype(MM_NP),
                "wk": np.ascontiguousarray(w_in[:, E : 2 * E][:, cols]).astype(MM_NP),
                "wv": np.ascontiguousarray(w_in[:, 2 * E :][:, cols]).astype(MM_NP),
                "bq": np.ascontiguousarray(b_in[:E][cols]).reshape(J, 1),
                "bk": np.ascontiguousarray(b_in[E : 2 * E][cols]).reshape(J, 1),
                "bv": np.broadcast_to(
                    b_in[2 * E :][cols].reshape(1, J), (128, J)
                ).copy(),
                "wout": np.ascontiguousarray(w_out[cols, :]).astype(MM_NP),
            }
        )
    return in_maps


def kernel(qkv, w_in, b_in, w_out, b_out, _trace=False):
    global _cached
    if _cached is None:
        _cached = _build()
    nc = _cached
    in_maps = _prep_inputs(qkv, w_in, b_in, w_out)
    res = bass_utils.run_bass_kernel_spmd(
        nc, in_maps, core_ids=list(range(NCORE)), trace=_trace
    )
    acc = np.zeros((S, E), np.float64)
    for r in res.results:
        acc += r["partial"].astype(np.float64)
    out = (acc + np.asarray(b_out, np.float32)[None, :]).astype(np.float32)
    out = out.reshape(1, S, E)
    if _trace:
        kernel.last_exec_time_ns = res.exec_time_ns
    return out
